# revision 22
# baseline (speedup 1.0000x reference)
"""Trainium2 Bass kernel: BasicMultiheadAttention (B=2, S=2048, D=1024, H=16).

Sharding: tensor-parallel over heads. Core c owns heads (2c, 2c+1) for both
batches; computes Q^T/K^T/V^T projections, attention in scores^T layout
(softmax exp on ACT, denominator via ones-augmented V in the PV matmul),
normalized ctx^T; per-query-chunk AllGather of ctx^T across the 8 cores; then
column-sharded output projection (+bias) per core.

PV runs in ctx^T orientation (V_aug stationary, P^T chunks moving, N=512); the
denominator lands in partition row 64, is copied to SBUF, transposed to
token-partition layout for a cheap strided reciprocal, broadcast back along
the free dim with a step-0 matmul against identity, and multiplied into ctx^T
on the DVE.

Host side: transpose/cast inputs to fp16, shard weights per core, re-assemble
the output transpose at the end.
"""

import numpy as np

B, S, D, H = 2, 2048, 1024, 16
DH = D // H  # 64
NCORES = 8
HPC = H // NCORES  # heads per core = 2
SQ = B * S  # 4096 tokens
NKT = D // 128  # 8 contraction k-tiles over D
KT_S = S // 128  # 16 key tiles per batch
QC_S = S // 512  # 4 query chunks of 512 per batch

_CACHE = {}


def _ensure_axon_hooks():
    """This image's antenv lacks axon_hooks; bass_utils imports it when
    trace=True under axon. Register an equivalent stub backed by the boot
    helper so NTFF profiling works (or degrades gracefully)."""
    import sys
    import types
    try:
        import antenv.axon_hooks  # noqa: F401
        return
    except ImportError:
        pass
    try:
        import antenv
        hook = [None]
        try:
            from trn_agent_boot.trn_boot import _ntff_profile_via_ctypes
            hook[0] = _ntff_profile_via_ctypes("/opt/axon/libaxon_pjrt.so")
        except Exception:
            hook[0] = None
        mod = types.ModuleType("antenv.axon_hooks")
        mod.get_axon_ntff_profile_hook = lambda: hook[0]
        mod.set_axon_ntff_profile_hook = lambda h: hook.__setitem__(0, h)
        sys.modules["antenv.axon_hooks"] = mod
        antenv.axon_hooks = mod
    except Exception:
        pass


_ensure_axon_hooks()


def _build_kernel():
    import concourse.bass as bass  # noqa: F401
    import concourse.mybir as mybir
    import concourse.tile as tile
    from concourse import bacc
    from concourse.masks import make_identity

    f16 = mybir.dt.float16
    f32 = mybir.dt.float32
    AF = mybir.ActivationFunctionType

    nc = bacc.Bacc(None, num_devices=NCORES)

    # ---- I/O ----
    xT = nc.dram_tensor("xT", [D, SQ], f16, kind="ExternalInput")
    wpack = nc.dram_tensor("wpack", [128, 4 * D], f16, kind="ExternalInput")
    bpack = nc.dram_tensor("bpack", [128, 4], f32, kind="ExternalInput")
    yT = nc.dram_tensor("yT", [128, SQ], f32, kind="ExternalOutput")

    with tile.TileContext(nc) as tc:
        with (
            tc.tile_pool(name="const", bufs=1) as const,
            tc.tile_pool(name="psA", bufs=2, space="PSUM") as psA,
            tc.tile_pool(name="psSc", bufs=2, space="PSUM") as psSc,
            tc.tile_pool(name="psCtx", bufs=2, space="PSUM") as psCtx,
            tc.tile_pool(name="pP", bufs=4) as pP,
            tc.tile_pool(name="pDen", bufs=4) as pDen,
            tc.tile_pool(name="pSmall", bufs=4) as pSmall,
            tc.tile_pool(name="pOut", bufs=2) as pOut,
            tc.tile_pool(name="dram", bufs=1, space="DRAM") as dram,
        ):
            # ---- small constants first (so QKV isn't gated on the big xT FIFO) ----
            wpack_sb = const.tile([128, 4 * D], f16)
            nc.sync.dma_start(wpack_sb[:], wpack[:, :])
            bpack_sb = const.tile([128, 4], f32)
            nc.sync.dma_start(bpack_sb[:], bpack[:, :])
            wq_sb = wpack_sb[:, 0 * D:1 * D]
            wk_sb = wpack_sb[:, 1 * D:2 * D]
            wv_sb = wpack_sb[:, 2 * D:3 * D]
            wo_sb = wpack_sb[:, 3 * D:4 * D]
            bq_sb = bpack_sb[:, 0:1]
            bk_sb = bpack_sb[:, 1:2]
            bvt_sb = bpack_sb[:, 2:3]
            bo_sb = bpack_sb[:, 3:4]
            ones_sb = const.tile([128, 1], f32)
            nc.vector.memset(ones_sb[:], 1.0)

            ident = const.tile([128, 128], f16)
            make_identity(nc, ident)
            ident32 = const.tile([128, 128], f32)
            make_identity(nc, ident32)

            xt_sb = const.tile([128, NKT * SQ], f16)
            for half in range(2):
                t0 = half * (SQ // 2)
                for kt in range(NKT):
                    nc.sync.dma_start(
                        xt_sb[:, kt * SQ + t0: kt * SQ + t0 + SQ // 2],
                        xT[kt * 128:(kt + 1) * 128, t0:t0 + SQ // 2],
                    )

            qT_sb = const.tile([128, SQ], f16)
            kT_sb = const.tile([128, SQ], f16)
            vT_sb = const.tile([128, SQ], f16)
            # V with ones column: per (b, head, key-tile) a [128, 65] region
            NREG = B * HPC * KT_S  # 64 regions
            vaug_sb = const.tile([128, NREG * 65], f16)
            ones_cols = vaug_sb.rearrange("p (r c) -> p r c", c=65)[:, :, 64:65]
            nc.vector.memset(ones_cols, 1.0)

            ctxT_sb = const.tile([128, SQ], f16)

            # warmup collective: absorb the first-trigger ncfw init delay early
            wu_loc = dram.tile([128, 2], f16, name="wu_loc")
            nc.sync.dma_start(wu_loc[:], ctxT_sb[:, 0:2])
            wu_g = dram.tile([NCORES * 128, 2], f16, addr_space="Shared", name="wu_g")
            nc.gpsimd.collective_compute(
                "AllGather", mybir.AluOpType.bypass,
                replica_groups=[list(range(NCORES))],
                ins=[wu_loc.opt()], outs=[wu_g.opt()],
            )

            # ---- QKV^T projections (all pipelined: weights stationary) ----
            for b in range(B):
                for (w_sb, b_sb, dst) in (
                    (wq_sb, bq_sb, qT_sb), (wk_sb, bk_sb, kT_sb), (wv_sb, bvt_sb, vT_sb),
                ):
                    for ncx in range(QC_S):
                        tok0 = b * S + ncx * 512
                        ps = psA.tile([128, 512], f32, tag="a", name=f"ps_{b}_{ncx}")
                        for kt in range(NKT):
                            nc.tensor.matmul(
                                ps[:],
                                lhsT=w_sb[:, kt * 128:(kt + 1) * 128],
                                rhs=xt_sb[:, kt * SQ + tok0: kt * SQ + tok0 + 512],
                                start=(kt == 0),
                                stop=(kt == NKT - 1),
                            )
                        nc.scalar.activation(
                            dst[:, tok0:tok0 + 512], ps[:], AF.Identity,
                            bias=b_sb,
                        )
                # V_aug: transpose V^T tiles into [tokens, dims] regions (+bias)
                for tt in range(KT_S):
                    tok0 = b * S + tt * 128
                    vtr = psA.tile([128, 128], f16, tag="a", name=f"vtr_{b}_{tt}")
                    nc.tensor.transpose(
                        vtr[:], vT_sb[:, tok0:tok0 + 128], ident[:],
                    )
                    for h in range(HPC):
                        r = (b * HPC + h) * KT_S + tt
                        nc.vector.tensor_copy(
                            vaug_sb[:, r * 65: r * 65 + 64],
                            vtr[:, h * 64:(h + 1) * 64],
                        )

            # ---- attention + per-qc AllGather + output projection ----
            pending_outproj = []
            for b in range(B):
                for qc in range(QC_S):
                    q0 = b * S + qc * 512
                    ctx_ps = [
                        psCtx.tile([65, 512], f32, tag="ctx", name=f"ctx_{b}_{qc}_{h}")
                        for h in range(HPC)
                    ]
                    for kt in range(KT_S):
                        k0 = b * S + kt * 128
                        sc = psSc.tile([128, 1024], f32, tag="sc", name=f"sc_{b}_{qc}_{kt}")
                        for h in range(HPC):
                            nc.tensor.matmul(
                                sc[:, h * 512:(h + 1) * 512],
                                lhsT=kT_sb[h * 64:(h + 1) * 64, k0:k0 + 128],
                                rhs=qT_sb[h * 64:(h + 1) * 64, q0:q0 + 512],
                                start=True,
                                stop=True,
                                tile_position=(h * 64, 0),
                            )
                        p_sb = pP.tile([128, 1024], f16, tag="p", name=f"p_{b}_{qc}_{kt}")
                        nc.scalar.activation(p_sb[:], sc[:], AF.Exp, scale=0.125)
                        # PV in ctx^T orientation: V_aug stationary, P chunk moving
                        for h in range(HPC):
                            r = (b * HPC + h) * KT_S + kt
                            nc.tensor.matmul(
                                ctx_ps[h][:],
                                lhsT=vaug_sb[:, r * 65:(r + 1) * 65],
                                rhs=p_sb[:, h * 512:(h + 1) * 512],
                                start=(kt == 0),
                                stop=(kt == KT_S - 1),
                            )
                    # normalize off the critical path: free the ctx psum via an
                    # SBUF copy, then denom -> recip -> broadcast -> multiply
                    for h in range(HPC):
                        ctxu = pDen.tile([128, 512], f32, tag="ctxu", name=f"ctxu_{b}_{qc}_{h}")
                        nc.vector.tensor_copy(ctxu[0:65, :], ctx_ps[h][:])
                        dtp = psCtx.tile([128, 512], f32, tag="ctx", name=f"dtp_{b}_{qc}_{h}")
                        for qs in range(4):
                            nc.tensor.transpose(
                                dtp[:, qs * 128:(qs + 1) * 128],
                                ctxu[:, qs * 128:(qs + 1) * 128],
                                ident32[:],
                            )
                        rec = pSmall.tile([128, 4], f16, tag="rec", name=f"rec_{b}_{qc}_{h}")
                        with nc.allow_low_precision(reason="softmax denom recip in fp16"):
                            nc.vector.reciprocal(
                                rec[:], dtp.rearrange("p (a c) -> p a c", c=128)[:, :, 64:65],
                            )
                        rps = psCtx.tile([64, 512], f32, tag="ctx", name=f"rps_{b}_{qc}_{h}")
                        for qs in range(4):
                            nc.tensor.matmul(
                                rps[:, qs * 128:(qs + 1) * 128],
                                lhsT=rec[:, qs:qs + 1].broadcast_to([128, 64]),
                                rhs=ident[:],
                                start=True,
                                stop=True,
                            )
                        nc.vector.tensor_mul(
                            ctxT_sb[h * 64:(h + 1) * 64, q0:q0 + 512],
                            ctxu[0:64, :],
                            rps[:],
                        )
                    # per-qc AllGather of ctx^T columns
                    ctx_loc = dram.tile([128, 512], f16, name=f"ctx_loc_{b}_{qc}")
                    nc.sync.dma_start(ctx_loc[:], ctxT_sb[:, q0:q0 + 512])
                    cg = dram.tile([NCORES * 128, 512], f16, addr_space="Shared",
                                   name=f"ctx_gath_{b}_{qc}")
                    nc.gpsimd.collective_compute(
                        "AllGather",
                        mybir.AluOpType.bypass,
                        replica_groups=[list(range(NCORES))],
                        ins=[ctx_loc.opt()],
                        outs=[cg.opt()],
                    )
                    # output projection for this qc, delayed by one qc so its
                    # AllGather wait sits behind the next chunk's attention in
                    # every in-order engine stream
                    def emit_outproj(b=b, qc=qc, q0=q0, cg=cg):
                        cgts = []
                        for kt in range(NKT):
                            cgt = pOut.tile([128, 512], f16, tag=f"cg{kt}",
                                            name=f"cg_{b}_{qc}_{kt}")
                            nc.sync.dma_start(cgt[:], cg[kt * 128:(kt + 1) * 128, :])
                            cgts.append(cgt)
                        po = psA.tile([128, 512], f32, tag="a", name=f"opp_{b}_{qc}")
                        for kt in range(NKT):
                            nc.tensor.matmul(
                                po[:],
                                lhsT=wo_sb[:, kt * 128:(kt + 1) * 128],
                                rhs=cgts[kt][:],
                                start=(kt == 0),
                                stop=(kt == NKT - 1),
                            )
                        out_sb = pOut.tile([128, 512], f32, tag="os", name=f"os_{b}_{qc}")
                        nc.vector.tensor_scalar_add(out_sb[:], po[:], bo_sb)
                        nc.sync.dma_start(yT[:, q0:q0 + 512], out_sb[:])

                    pending_outproj.append(emit_outproj)
                    if len(pending_outproj) > 3:
                        pending_outproj.pop(0)()

            for fn in pending_outproj:
                fn()

    nc.finalize()
    return nc


def kernel(x, Wq, Wk, Wv, bq, bk, bv, Wo, bo):
    from concourse.bass_utils import run_bass_kernel_spmd

    if "nc" not in _CACHE:
        _CACHE["nc"] = _build_kernel()
    nc = _CACHE["nc"]

    # host-side prep
    xTh = np.ascontiguousarray(
        x.astype(np.float32).transpose(2, 0, 1).reshape(D, SQ)
    ).astype(np.float16)

    def pack_w(Wslice):
        # [D, 128] -> [128, D] kt-major: out[p, kt*128+m] = Wslice[kt*128+p, m]
        return np.ascontiguousarray(
            Wslice.reshape(NKT, 128, 128).transpose(1, 0, 2).reshape(128, D)
        ).astype(np.float16)

    in_maps = []
    for c in range(NCORES):
        hA, hB = HPC * c, HPC * c + 1
        wq_c = pack_w(np.concatenate([Wq[hA], Wq[hB]], axis=1))
        wk_c = pack_w(np.concatenate([Wk[hA], Wk[hB]], axis=1))
        wv_c = pack_w(np.concatenate([Wv[hA], Wv[hB]], axis=1))
        wo_c = pack_w(Wo[:, 128 * c:128 * (c + 1)])
        wpack_c = np.ascontiguousarray(
            np.concatenate([wq_c, wk_c, wv_c, wo_c], axis=1))
        bq_c = np.concatenate([bq[hA], bq[hB]]).reshape(128, 1)
        bk_c = np.concatenate([bk[hA], bk[hB]]).reshape(128, 1)
        bv_c = np.concatenate([bv[hA], bv[hB]]).reshape(128, 1)
        bo_c = bo[128 * c:128 * (c + 1)].reshape(128, 1)
        bpack_c = np.ascontiguousarray(
            np.concatenate([bq_c, bk_c, bv_c, bo_c], axis=1)).astype(np.float32)
        in_maps.append({"xT": xTh, "wpack": wpack_c, "bpack": bpack_c})

    res = run_bass_kernel_spmd(nc, in_maps, core_ids=list(range(NCORES)))
    _CACHE["last_result"] = res
    # assemble: core c's yT [128, SQ] are output columns 128c..128c+127 (transposed)
    out = np.empty((B, S, D), dtype=np.float32)
    for c in range(NCORES):
        yt = res.results[c]["yT"]  # [128, SQ]
        out[:, :, 128 * c:128 * (c + 1)] = (
            yt.reshape(128, B, S).transpose(1, 2, 0)
        )
    return out
